# revision 4
# baseline (speedup 1.0000x reference)
"""Multi-head latent attention (MLA) TRN2 kernel — v2.

Sharding: batch(2) x query-sequence(4) over 8 cores (same as v1). Each core
computes the full KV path for its batch, the Q path + attention + o_proj for
its 512-token query chunk. Host assembles slices. No collectives.

v2 speedups over the f32r baseline:
- Projection matmuls (q_a, q_b, kv_a, kv_b) run as 3-term split-fp8
  (hi+lo e4m3) DoubleRow matmuls: 2 K-tiles per instruction at 0.5
  cycles/row -> 0.75x the PE cycles of f32r at ~bf16-level accuracy.
  x and all fp8 weights are pre-split hi/lo on the host (x*4, W*64).
- Everything else (scores, PV, o_proj, tables) in bf16: same PE speed as
  f32r but half the DMA/SBUF and 2-4x faster DVE ops.
- All intermediates SBUF-resident (no DRAM spills/reloads).
- RMSNorm folding: ln weights folded into q_b/kv_b rows on host; the
  1/rms per-token factors folded into the consumers' psum->sbuf copies
  (broadcast row for [feat,tok] outputs, per-partition scalar for
  [tok,feat] outputs); psum descale factors folded into the rsqrt
  activation's scale/bias and the k-rope cos/sin tables.
- Softmax denominator accumulated as bf16 tensor adds (4x DVE mode) and
  reduced across partitions with one ones-matmul per head.
- Weight streams issued on the gpsimd (Pool/SWDGE) DMA queue so they
  prefetch across phase boundaries without blocking the sync queue.
"""

import math

import numpy as np
import ml_dtypes

B, T, HID = 2, 2048, 2048
NH, NKV = 16, 8
NOPE, ROPE = 128, 64
HD = NOPE + ROPE  # 192
VD = 128
KV_RANK, Q_RANK = 512, 1536
EPS = 1e-6
THETA = 10000.0
NCORES = 8
TQ = B * T // NCORES  # 512
P = 128
SCALE = 1.0 / math.sqrt(HD)
SX = 4.0  # fp8 scale for hidden_states
SW = 64.0  # fp8 scale for weights
S1 = SX * SW  # scale carried by a-proj psums (256)
SQBIAS = EPS * S1 * S1  # rsqrt bias absorbing the psum scale

F8NP = ml_dtypes.float8_e4m3
BFNP = ml_dtypes.bfloat16

_CACHE = {}


def _build_nc():
    import concourse.bass as bass  # noqa: F401
    import concourse.mybir as mybir
    from concourse import bacc
    from concourse.tile import TileContext

    F32 = mybir.dt.float32
    BF = mybir.dt.bfloat16
    F8 = mybir.dt.float8e4
    AF = mybir.ActivationFunctionType
    ALU = mybir.AluOpType
    DR = mybir.MatmulPerfMode.DoubleRow

    nc = bacc.Bacc(None, target_bir_lowering=False)

    def din(name, shape, dt):
        return nc.dram_tensor(name, shape, dt, kind="ExternalInput")

    # all streamed tensors are packed tile-major on the host so every DMA
    # is 128 long contiguous runs (one per partition), not a 2048-descriptor
    # gather.
    xT_hi = din("xT_hi", [P, 8, 16, 256], F8)   # [p, chunk, kt, c]
    xT_lo = din("xT_lo", [P, 8, 16, 256], F8)
    xq_hi = din("xq_hi", [P, 16, TQ], F8)       # [p, kt, t]
    xq_lo = din("xq_lo", [P, 16, TQ], F8)
    qa_hi = din("qa_hi", [P, 12, 16, 128], F8)  # [p, m, kt, c]
    qa_lo = din("qa_lo", [P, 12, 16, 128], F8)
    qb_hi = din("qb_hi", [P, 24, 12, 128], F8)  # [p, m, kt, c]
    qb_lo = din("qb_lo", [P, 24, 12, 128], F8)
    kva_hi = din("kva_hi", [P, 8, 16, 128], F8)  # [p, m, kt, c]
    kva_lo = din("kva_lo", [P, 8, 16, 128], F8)
    kvb_hi = din("kvb_hi", [P, 8, 4, 256], F8)   # [p, slice(4 nope+4 v), kt, c]
    kvb_lo = din("kvb_lo", [P, 8, 4, 256], F8)
    ow_hi = din("ow_hi", [P, 4, 16, 512], F8)    # [p, n, ht, c]
    ow_lo = din("ow_lo", [P, 4, 16, 512], F8)
    cosq = din("cosq", [P, TQ], BF)
    sinq = din("sinq", [P, TQ], BF)
    cosk = din("cosk", [P, T], BF)  # pre-divided by S1
    sink = din("sink", [P, T], BF)
    ones_in = din("ones_in", [P, P], BF)
    ones32_in = din("ones32_in", [P, P], BF)  # 1/32: scales attn into fp8 range
    consts = din("consts", [P, 2], F32)  # [1/SW, SQBIAS]
    out = nc.dram_tensor("out", [TQ, HID], F32, kind="ExternalOutput")


    with TileContext(nc) as tc:
        with (
            tc.tile_pool(name="tbl", bufs=1) as tbl,
            tc.tile_pool(name="big", bufs=1) as big,
            tc.tile_pool(name="wkv", bufs=2) as wkv,  # kvb weights (gpsimd)
        ):
            # --- persistent tables / intermediates
            ones_sb = tbl.tile([P, P], BF, name="ones_sb")
            ones32_sb = tbl.tile([P, P], BF, name="ones32_sb")
            consts_sb = tbl.tile([P, 2], F32, name="consts_sb")
            inv_sw = consts_sb[:, 0:1]
            sqbias = consts_sb[:, 1:2]

            klat_hi = big.tile([P, 4, T], F8, name="klat_hi")
            klat_lo = big.tile([P, 4, T], F8, name="klat_lo")
            krp8 = big.tile([P, 2, 2, T], F8, name="krp8")
            qnoped = big.tile([P, NH, TQ], BF, name="qnoped")
            qp8 = big.tile([P, 4, 2, TQ], F8, name="qp8")
            rsb1 = big.tile([P, T], BF, name="rsb1")
            rsb2b = big.tile([P, 16], BF, name="rsb2b")
            rsb2 = big.tile([P, 16], F32, name="rsb2")

            # ---------------- P1 + P2 shared scope -------------------------
            with (
                tc.tile_pool(name="early", bufs=1) as early,
                tc.tile_pool(name="wq", bufs=2) as wq,  # qa weights (sync)
            ):
                cosq_sb = early.tile([P, TQ], BF, name="cosq_sb")
                sinq_sb = early.tile([P, TQ], BF, name="sinq_sb")
                xqh_sb = early.tile([P, 16, TQ], F8, name="xqh_sb")
                xql_sb = early.tile([P, 16, TQ], F8, name="xql_sb")

                def load_qa(m):
                    h = wq.tile([P, 16, P], F8, tag="qa_h")
                    nc.sync.dma_start(h[:], qa_hi[:, m])
                    l = wq.tile([P, 16, P], F8, tag="qa_l")
                    nc.sync.dma_start(l[:], qa_lo[:, m])
                    return h, l

                qa_q = []

                # ------------- P1: kv_a + rmsnorm + rope -------------------
                NCH = 8
                CW = T // NCH  # 256
                with (
                    tc.tile_pool(name="p1", bufs=1) as p1,
                    tc.tile_pool(name="p1s", bufs=2) as p1s,
                    tc.tile_pool(name="p1t", bufs=1) as p1t,
                    tc.tile_pool(name="p1ps", bufs=3, space="PSUM") as psA,
                    tc.tile_pool(name="p1ps1", bufs=2, space="PSUM") as psS,
                ):
                    # rotated rope halves, half-T at a time (scatter twice).
                    # allocated in `early` so P2's tiles don't alias them (the
                    # scatter DMAs read them on the slow Pool queue).
                    rotH1 = early.tile([P, 2, T], F8, name="rotH1")
                    rotH2 = early.tile([P, 2, T], F8, name="rotH2")

                    def scatter_k():
                        for tt in range(2):
                            for i in range(4):
                                kvh = 4 * tt + i
                                bb = 32 * (kvh % 4)
                                nc.gpsimd.dma_start(
                                    krp8[bb : bb + 32, kvh // 4, 0, :],
                                    rotH1[i * 32 : (i + 1) * 32, tt, :],
                                )
                                nc.gpsimd.dma_start(
                                    krp8[bb : bb + 32, kvh // 4, 1, :],
                                    rotH2[i * 32 : (i + 1) * 32, tt, :],
                                )

                    def load_xch(nch):
                        h = p1s.tile([P, 16, CW], F8, tag="xch_h")
                        nc.sync.dma_start(h[:], xT_hi[:, nch])
                        l = p1s.tile([P, 16, CW], F8, tag="xch_l")
                        nc.sync.dma_start(l[:], xT_lo[:, nch])
                        return h, l

                    # kv_a weights: first m-slice on the fast sync queue so
                    # P1 starts immediately; rest on gpsimd
                    kvaw_h = p1.tile([P, 8, 16, 128], F8, name="kvaw_h")
                    kvaw_l = p1.tile([P, 8, 16, 128], F8, name="kvaw_l")
                    nc.sync.dma_start(kvaw_h[:, 0], kva_hi[:, 0])
                    cosk_sb = p1.tile([P, T], BF, name="cosk_sb")
                    sink_sb = p1.tile([P, T], BF, name="sink_sb")
                    xch0_h = p1s.tile([P, 16, CW], F8, tag="xch_h")
                    nc.scalar.dma_start(xch0_h[:, 0:8, :], xT_hi[:, 0, 0:8])
                    nc.sync.dma_start(kvaw_l[:, 0], kva_lo[:, 0])
                    nc.scalar.dma_start(xch0_h[:, 8:16, :], xT_hi[:, 0, 8:16])
                    xch0_l = p1s.tile([P, 16, CW], F8, tag="xch_l")
                    nc.sync.dma_start(xch0_l[:], xT_lo[:, 0])
                    nc.scalar.dma_start(kvaw_h[:, 1:4], kva_hi[:, 1:4])
                    nc.sync.dma_start(kvaw_l[:, 1:4], kva_lo[:, 1:4])
                    nc.gpsimd.dma_start(kvaw_h[:, 4:8], kva_hi[:, 4:8])
                    nc.gpsimd.dma_start(kvaw_l[:, 4:8], kva_lo[:, 4:8])
                    nc.scalar.dma_start(ones_sb[:], ones_in[:, :])
                    nc.scalar.dma_start(ones32_sb[:], ones32_in[:, :])
                    nc.scalar.dma_start(consts_sb[:], consts[:, :])
                    nxt = (xch0_h, xch0_l)
                    qa_q.extend(load_qa(m) for m in range(2))
                    # tables / P2 inputs queued behind the critical first loads
                    nc.sync.dma_start(cosk_sb[:], cosk[:, :])
                    nc.sync.dma_start(sink_sb[:], sink[:, :])
                    for nch in range(NCH):
                        chsl = slice(nch * CW, (nch + 1) * CW)
                        xch_h, xch_l = nxt
                        if nch + 1 < NCH:
                            nxt = load_xch(nch + 1)
                        if nch == 4:
                            nc.sync.dma_start(cosq_sb[:], cosq[:, :])
                            nc.sync.dma_start(sinq_sb[:], sinq[:, :])
                        elif nch == 5:
                            nc.sync.dma_start(xqh_sb[:], xq_hi[:, :, :])
                        elif nch == 6:
                            nc.sync.dma_start(xql_sb[:], xq_lo[:, :, :])
                        sumsq = psS.tile([P, CW], F32, tag="k_sumsq")
                        raw1 = p1s.tile([P, 2, CW], BF, tag="k_raw1")
                        raw2 = p1s.tile([P, 2, CW], BF, tag="k_raw2")
                        for m in range(8):
                            ps = psA.tile([P, CW], F32, tag="kva_ps")
                            first = True
                            for wt, xt in (
                                (kvaw_h, xch_h),
                                (kvaw_h, xch_l),
                                (kvaw_l, xch_h),
                            ):
                                for j in range(8):
                                    nc.tensor.matmul(
                                        ps[:],
                                        wt[:, m, 2 * j : 2 * j + 2, :],
                                        xt[:, 2 * j : 2 * j + 2, :],
                                        start=first,
                                        stop=(wt is kvaw_l) and j == 7,
                                        perf_mode=DR,
                                    )
                                    first = False
                            if m < 4:
                                # latent: quantize hi/lo + square for sumsq
                                nc.scalar.activation(
                                    klat_hi[:, m, chsl], ps[:], AF.Copy,
                                    scale=1.0 / SW,
                                )
                                nc.vector.scalar_tensor_tensor(
                                    klat_lo[:, m, chsl], ps[:], inv_sw,
                                    klat_hi[:, m, chsl], ALU.mult, ALU.subtract,
                                )
                                sq = p1s.tile([P, CW], BF, tag="k_sq")
                                nc.scalar.square(sq[:], ps[:])
                                nc.tensor.matmul(
                                    sumsq[:], ones_sb[:], sq[:],
                                    start=(m == 0), stop=(m == 3),
                                )
                            elif m < 6:
                                nc.scalar.copy(raw1[:, m - 4, :], ps[:])
                            else:
                                nc.scalar.copy(raw2[:, m - 6, :], ps[:])
                        sqt = p1t.tile([P, CW], F32, tag="k_sqt")
                        nc.scalar.activation(
                            sqt[:], sumsq[:], AF.Sqrt,
                            scale=1.0 / KV_RANK, bias=sqbias,
                        )
                        with nc.allow_low_precision(reason="bf16 rms factor ok at 2e-2 tol"):
                            nc.vector.reciprocal(rsb1[:, chsl], sqt[:])
                        # transposed rs for the vp copies ([tok] on partitions)
                        for cc in range(2):
                            tok0 = nch * CW + cc * P
                            nc.gpsimd.dma_start(
                                rsb2b[:, 2 * nch + cc : 2 * nch + cc + 1],
                                rsb1[0:1, tok0 : tok0 + P],
                            )
                        # rotate this chunk's rope rows into rotH1/rotH2
                        for tt in range(2):
                            tmp1 = p1t.tile([P, CW], BF, tag="k_rtmp1")
                            tmp2 = p1t.tile([P, CW], BF, tag="k_rtmp2")
                            nc.vector.tensor_tensor(
                                tmp1[:], raw2[:, tt, :], sink_sb[:, chsl], ALU.mult
                            )
                            nc.vector.tensor_tensor(
                                tmp2[:], raw1[:, tt, :], sink_sb[:, chsl], ALU.mult
                            )
                            nc.vector.tensor_tensor(
                                raw1[:, tt, :], raw1[:, tt, :], cosk_sb[:, chsl],
                                ALU.mult,
                            )
                            nc.vector.tensor_tensor(
                                rotH1[:, tt, chsl], raw1[:, tt, :], tmp1[:],
                                ALU.subtract,
                            )
                            nc.vector.tensor_tensor(
                                raw2[:, tt, :], raw2[:, tt, :], cosk_sb[:, chsl],
                                ALU.mult,
                            )
                            nc.vector.tensor_tensor(
                                rotH2[:, tt, chsl], raw2[:, tt, :], tmp2[:],
                                ALU.add,
                            )
                    nc.vector.tensor_copy(rsb2[:], rsb2b[:])
                    scatter_k()

                # kvb weights: rolling 2-deep prefetch on the gpsimd queue
                def load_kvb(hp):
                    wn_h = wkv.tile([P, 4, 256], F8, tag="wn_h")
                    nc.gpsimd.dma_start(wn_h[:], kvb_hi[:, hp])
                    wn_l = wkv.tile([P, 4, 256], F8, tag="wn_l")
                    nc.gpsimd.dma_start(wn_l[:], kvb_lo[:, hp])
                    wv_h = wkv.tile([P, 4, 256], F8, tag="wv_h")
                    nc.gpsimd.dma_start(wv_h[:], kvb_hi[:, 4 + hp])
                    wv_l = wkv.tile([P, 4, 256], F8, tag="wv_l")
                    nc.gpsimd.dma_start(wv_l[:], kvb_lo[:, 4 + hp])
                    return (wn_h, wn_l, wv_h, wv_l)

                kvb_tiles = [load_kvb(0), load_kvb(1)]

                # ------------- P2: q path ----------------------------------
                with (
                    tc.tile_pool(name="p2", bufs=1) as p2,
                    tc.tile_pool(name="wqb", bufs=4) as wqb,
                    tc.tile_pool(name="p2s", bufs=2) as p2s,
                    tc.tile_pool(name="p2ps", bufs=2, space="PSUM") as psA,
                    tc.tile_pool(name="p2ps1", bufs=1, space="PSUM") as psS,
                ):
                    ql_hi = p2.tile([P, 12, TQ], F8, name="ql_hi")
                    ql_lo = p2.tile([P, 12, TQ], F8, name="ql_lo")
                    recq = p2.tile([P, TQ], BF, name="recq")
                    sumsq = psS.tile([P, TQ], F32, tag="q_sumsq")
                    for m in range(12):
                        wt_h, wt_l = qa_q.pop(0)
                        if m + 2 < 12:
                            qa_q.append(load_qa(m + 2))
                        ps = psA.tile([P, TQ], F32, tag="qa_ps")
                        first = True
                        for wt, xt in (
                            (wt_h, xqh_sb),
                            (wt_h, xql_sb),
                            (wt_l, xqh_sb),
                        ):
                            for j in range(8):
                                nc.tensor.matmul(
                                    ps[:],
                                    wt[:, 2 * j : 2 * j + 2, :],
                                    xt[:, 2 * j : 2 * j + 2, :],
                                    start=first,
                                    stop=(wt is wt_l) and j == 7,
                                    perf_mode=DR,
                                )
                                first = False
                        nc.scalar.activation(
                            ql_hi[:, m, :], ps[:], AF.Copy, scale=1.0 / SW
                        )
                        nc.vector.scalar_tensor_tensor(
                            ql_lo[:, m, :], ps[:], inv_sw, ql_hi[:, m, :],
                            ALU.mult, ALU.subtract,
                        )
                        sq = p2s.tile([P, TQ], BF, tag="q_sq")
                        nc.scalar.square(sq[:], ps[:])
                        nc.tensor.matmul(
                            sumsq[:], ones_sb[:], sq[:],
                            start=(m == 0), stop=(m == 11),
                        )
                    sqt = p2s.tile([P, TQ], F32, tag="q_sqt")
                    nc.scalar.activation(
                        sqt[:], sumsq[:], AF.Sqrt,
                        scale=1.0 / Q_RANK, bias=sqbias,
                    )
                    with nc.allow_low_precision(reason="bf16 rms factor ok at 2e-2 tol"):
                        nc.vector.reciprocal(recq[:], sqt[:])

                    def qb_mm(m, ps):
                        wt_h = wqb.tile([P, 12, P], F8, tag="qb_h")
                        nc.sync.dma_start(wt_h[:], qb_hi[:, m])
                        wt_l = wqb.tile([P, 12, P], F8, tag="qb_l")
                        nc.sync.dma_start(wt_l[:], qb_lo[:, m])
                        first = True
                        for wt, xt in (
                            (wt_h, ql_hi),
                            (wt_h, ql_lo),
                            (wt_l, ql_hi),
                        ):
                            for j in range(6):
                                nc.tensor.matmul(
                                    ps[:],
                                    wt[:, 2 * j : 2 * j + 2, :],
                                    xt[:, 2 * j : 2 * j + 2, :],
                                    start=first,
                                    stop=(wt is wt_l) and j == 5,
                                    perf_mode=DR,
                                )
                                first = False

                    # rope tiles first (heads needed early in P3)
                    qraw1 = p2.tile([P, 4, TQ], BF, name="qraw1")
                    qraw2 = p2.tile([P, 4, TQ], BF, name="qraw2")
                    for m in range(16, 24):
                        ps = psA.tile([P, TQ], F32, tag="qb_ps")
                        qb_mm(m, ps)
                        dst = qraw1 if m < 20 else qraw2
                        nc.vector.tensor_tensor(
                            dst[:, m % 4, :], ps[:], recq[:], ALU.mult
                        )
                    # rotation (in-place on qraw; gpsimd scatter casts to fp8)
                    cb = cosq_sb[:, None, :].to_broadcast((P, 2, TQ))
                    sb = sinq_sb[:, None, :].to_broadcast((P, 2, TQ))
                    tmp1 = p2.tile([P, 2, TQ], BF, name="q_rtmp1")
                    tmp2 = p2.tile([P, 2, TQ], BF, name="q_rtmp2")
                    for g in range(2):
                        gs = slice(2 * g, 2 * g + 2)
                        nc.vector.tensor_tensor(tmp1[:], qraw2[:, gs, :], sb, ALU.mult)
                        nc.vector.tensor_tensor(tmp2[:], qraw1[:, gs, :], sb, ALU.mult)
                        nc.vector.tensor_tensor(qraw1[:, gs, :], qraw1[:, gs, :], cb, ALU.mult)
                        nc.vector.tensor_tensor(qraw1[:, gs, :], qraw1[:, gs, :], tmp1[:], ALU.subtract)
                        nc.vector.tensor_tensor(qraw2[:, gs, :], qraw2[:, gs, :], cb, ALU.mult)
                        nc.vector.tensor_tensor(qraw2[:, gs, :], qraw2[:, gs, :], tmp2[:], ALU.add)
                    # nope tiles; q-scatters interleaved (sync queue) so the
                    # first qpaird tiles land before P3's first scores
                    def scatter_q(h):
                        tau = (h % 2) + 2 * (h // 8)
                        bb = 32 * ((h // 2) % 4)
                        nc.gpsimd.dma_start(
                            qp8[bb : bb + 32, tau, 0, :],
                            qraw1[(h % 4) * 32 : (h % 4) * 32 + 32, h // 4, :],
                        )
                        nc.gpsimd.dma_start(
                            qp8[bb : bb + 32, tau, 1, :],
                            qraw2[(h % 4) * 32 : (h % 4) * 32 + 32, h // 4, :],
                        )

                    for m in range(16):
                        if m < 4:
                            for h in (4 * m, 4 * m + 1, 4 * m + 2, 4 * m + 3):
                                scatter_q(h)
                        ps = psA.tile([P, TQ], F32, tag="qb_ps")
                        qb_mm(m, ps)
                        nc.vector.tensor_tensor(
                            qnoped[:, m, :], ps[:], recq[:], ALU.mult
                        )

            # ------------- P3: attention -----------------------------------
            with (
                tc.tile_pool(name="p4w", bufs=2) as p4w,
                tc.tile_pool(name="p3", bufs=1) as p3,
                tc.tile_pool(name="p3q", bufs=2) as p3q,
                tc.tile_pool(name="p3p", bufs=3) as p3p,
                tc.tile_pool(name="scps", bufs=3, space="PSUM") as scps,
                tc.tile_pool(name="atps", bufs=2, space="PSUM") as atps,
                tc.tile_pool(name="prps", bufs=2, space="PSUM") as prps,
                tc.tile_pool(name="dnps", bufs=1, space="PSUM") as dnps,
            ):
                pending = []

                def finalize(item):
                    dsum, at, qh = item
                    dn = dnps.tile([P, TQ], F32, tag="dn")
                    nc.tensor.matmul(dn[:], ones_sb[:], dsum[:], start=True, stop=True)
                    rec = p3q.tile([P, TQ], F32, tag="rec")
                    nc.vector.reciprocal(rec[:], dn[:])
                    nc.vector.tensor_tensor(
                        attn_sb[:, qh, :], at[:], rec[:], ALU.mult
                    )

                for hp in range(4):  # kv-head pairs
                    kvh0 = 2 * hp
                    wn_h = wkv.tile([P, 4, 256], F8, tag="wn_h")
                    nc.gpsimd.dma_start(wn_h[:], kvb_hi[:, hp])
                    wn_l = wkv.tile([P, 4, 256], F8, tag="wn_l")
                    nc.gpsimd.dma_start(wn_l[:], kvb_lo[:, hp])
                    wv_h = wkv.tile([P, 4, 256], F8, tag="wv_h")
                    nc.gpsimd.dma_start(wv_h[:], kvb_hi[:, 4 + hp])
                    wv_l = wkv.tile([P, 4, 256], F8, tag="wv_l")
                    nc.gpsimd.dma_start(wv_l[:], kvb_lo[:, 4 + hp])

                    knp = p3.tile([P, 2, T], BF, tag="knp")
                    for h2 in range(2):
                        hsl = slice(h2 * P, (h2 + 1) * P)
                        for nch in range(4):
                            csl = slice(nch * 512, (nch + 1) * 512)
                            ps = prps.tile([P, 512], F32, tag="pr_ps")
                            first = True
                            for wt, lt in (
                                (wn_h, klat_hi),
                                (wn_h, klat_lo),
                                (wn_l, klat_hi),
                            ):
                                for j in range(2):
                                    nc.tensor.matmul(
                                        ps[:],
                                        wt[:, 2 * j : 2 * j + 2, hsl],
                                        lt[:, 2 * j : 2 * j + 2, csl],
                                        start=first,
                                        stop=(wt is wn_l) and j == 1,
                                        perf_mode=DR,
                                    )
                                    first = False
                            nc.vector.tensor_tensor(
                                knp[:, h2, csl], ps[:], rsb1[:, csl], ALU.mult
                            )
                    vp = p3.tile([P, 16, 256], BF, tag="vp")
                    for mt in range(16):
                        msl = slice(mt * P, (mt + 1) * P)
                        psf = prps.tile([P, 512], F32, tag="pr_ps")
                        ps = psf[:, :256]
                        first = True
                        for lt, wt in (
                            (klat_hi, wv_h),
                            (klat_hi, wv_l),
                            (klat_lo, wv_h),
                        ):
                            for j in range(2):
                                nc.tensor.matmul(
                                    ps[:],
                                    lt[:, 2 * j : 2 * j + 2, msl],
                                    wt[:, 2 * j : 2 * j + 2, :],
                                    start=first,
                                    stop=(lt is klat_lo) and j == 1,
                                    perf_mode=DR,
                                )
                                first = False
                        nc.scalar.activation(
                            vp[:, mt, :], ps[:], AF.Copy, scale=rsb2[:, mt : mt + 1]
                        )

                    for j4 in range(4):
                        qh = 4 * hp + j4
                        kvh = qh // 2
                        h2 = kvh - kvh0
                        qn = qnoped[:, qh, :]
                        b32 = 32 * ((qh // 2) % 4)
                        tau = (qh % 2) + 2 * (qh // 8)
                        qp = qp8[b32 : b32 + 32, tau, :, :]
                        krp = krp8[b32 : b32 + 32, kvh // 4, :, :]
                        dsum = p3q.tile([P, TQ], BF, tag="dsum")
                        at = atps.tile([P, TQ], F32, tag="at")
                        pts = {}
                        for kt in range(16):
                            ksl = slice(kt * P, (kt + 1) * P)
                            sc = scps.tile([P, TQ], F32, tag="sc")
                            nc.tensor.matmul(
                                sc[:], knp[:, h2, ksl], qn, start=True, stop=False
                            )
                            nc.tensor.matmul(
                                sc[:],
                                krp[:, :, ksl],
                                qp[:, :, :],
                                start=False,
                                stop=True,
                                perf_mode=DR,
                                skip_group_check=True,
                            )
                            pt = p3p.tile([P, TQ], BF, tag="probsT")
                            nc.scalar.activation(pt[:], sc[:], AF.Exp, scale=float(SCALE))
                            pts[kt] = pt
                            if kt == 0:
                                nc.vector.tensor_copy(dsum[:], pt[:])
                            else:
                                nc.vector.tensor_tensor(
                                    dsum[:], dsum[:], pt[:], ALU.add
                                )
                            if kt > 0:  # PV one stage behind scores
                                nc.tensor.matmul(
                                    at[:],
                                    vp[:, kt - 1, h2 * P : (h2 + 1) * P],
                                    pts[kt - 1][:],
                                    start=(kt == 1),
                                    stop=False,
                                )
                                del pts[kt - 1]
                        nc.tensor.matmul(
                            at[:],
                            vp[:, 15, h2 * P : (h2 + 1) * P],
                            pts[15][:],
                            start=False,
                            stop=True,
                        )
                        pending.append((dsum, at, qh))
                        if len(pending) == 2:
                            finalize(pending.pop(0))
                while pending:
                    finalize(pending.pop(0))

            # ------------- P4: o_proj --------------------------------------
            with (
                tc.tile_pool(name="p4s", bufs=2) as p4s,
                tc.tile_pool(name="p4ps", bufs=2, space="PSUM") as p4ps,
            ):
                for n in range(4):
                    ow = p4w.tile([P, 16, 512], BF, tag="ow")
                    nc.gpsimd.dma_start(ow[:], o_w[:, n])
                    for mt in range(4):
                        msl = slice(mt * P, (mt + 1) * P)
                        ps = p4ps.tile([P, 512], F32, tag="o_ps")
                        for h in range(NH):
                            nc.tensor.matmul(
                                ps[:],
                                attn_sb[:, h, msl],
                                ow[:, h, :],
                                start=(h == 0),
                                stop=(h == 15),
                            )
                        st = p4s.tile([P, 512], F32, tag="ost")
                        nc.scalar.copy(st[:], ps[:])
                        nc.sync.dma_start(
                            out[mt * P : (mt + 1) * P, n * 512 : (n + 1) * 512], st[:]
                        )

    nc.finalize()
    return nc


def _split8(a, s):
    hi = np.asarray(a * s, np.float32).astype(F8NP)
    lo = (np.asarray(a * s, np.float32) - hi.astype(np.float32)).astype(F8NP)
    return hi, lo


def _pack(w, c):
    """[K, M] row-major -> [P, M//c, K//P, c] tile-major (contiguous per
    partition for each (m-tile) DMA)."""
    K, M = w.shape
    kt = K // P
    nm = M // c
    return np.ascontiguousarray(
        w.reshape(kt, P, nm, c).transpose(1, 2, 0, 3)
    )


def _host_prep(inputs):
    x = np.asarray(inputs["hidden_states"], dtype=np.float32)
    lnq = np.asarray(inputs["q_a_ln_w"], np.float32)
    lnkv = np.asarray(inputs["kv_a_ln_w"], np.float32)

    qa_hi, qa_lo = _split8(np.asarray(inputs["q_a_w"], np.float32), SW)
    qa_hi, qa_lo = _pack(qa_hi, 128), _pack(qa_lo, 128)

    qb = np.asarray(inputs["q_b_w"], np.float32) * lnq[:, None]
    qb = qb.reshape(Q_RANK, NH, HD)
    nope_cols = qb[:, :, :NOPE].reshape(Q_RANK, NH * NOPE)
    rope1 = qb[:, :, NOPE : NOPE + 32].reshape(Q_RANK, 16 * 32)
    rope2 = qb[:, :, NOPE + 32 :].reshape(Q_RANK, 16 * 32)
    qb_hi, qb_lo = _split8(np.concatenate([nope_cols, rope1, rope2], axis=1), SW)
    qb_hi, qb_lo = _pack(qb_hi, 128), _pack(qb_lo, 128)

    kva = np.asarray(inputs["kv_a_w"], np.float32)
    lat = kva[:, :KV_RANK]
    krope = kva[:, KV_RANK:].reshape(HID, NKV, ROPE)
    kr1 = krope[:, :, :32].reshape(HID, NKV * 32)
    kr2 = krope[:, :, 32:].reshape(HID, NKV * 32)
    kva_hi, kva_lo = _split8(np.concatenate([lat, kr1, kr2], axis=1), SW)
    kva_hi, kva_lo = _pack(kva_hi, 128), _pack(kva_lo, 128)

    kvb = np.asarray(inputs["kv_b_w"], np.float32) * lnkv[:, None]
    kvb = kvb.reshape(KV_RANK, NKV, NOPE + VD)
    knope_cols = kvb[:, :, :NOPE].reshape(KV_RANK, NKV * NOPE)
    v_cols = kvb[:, :, NOPE:].reshape(KV_RANK, NKV * VD)
    kvb_hi, kvb_lo = _split8(np.concatenate([knope_cols, v_cols], axis=1), SW)
    kvb_hi, kvb_lo = _pack(kvb_hi, 256), _pack(kvb_lo, 256)

    ow_hi, ow_lo = _split8(np.asarray(inputs["o_w"], np.float32), SW)
    ow_hi = _pack_f8(ow_hi, 512).reshape(P, 4, 16, 512)
    ow_lo = _pack_f8(ow_lo, 512).reshape(P, 4, 16, 512)

    inv_freq = 1.0 / (THETA ** (np.arange(0, ROPE, 2, dtype=np.float32) / ROPE))
    t = np.arange(T, dtype=np.float32)
    freqs = np.outer(t, inv_freq).astype(np.float32)
    cosq_full = np.tile(np.cos(freqs).T, (4, 1)).astype(BFNP)  # [128, T]
    sinq_full = np.tile(np.sin(freqs).T, (4, 1)).astype(BFNP)
    cosk = (np.tile(np.cos(freqs).T, (4, 1)) / S1).astype(BFNP)
    sink = (np.tile(np.sin(freqs).T, (4, 1)) / S1).astype(BFNP)
    ones = np.ones((P, P), BFNP)
    ones32 = np.full((P, P), 1.0 / 32.0, BFNP)
    consts = np.empty((P, 2), np.float32)
    consts[:, 0] = 1.0 / SW
    consts[:, 1] = SQBIAS

    in_maps = []
    for c in range(NCORES):
        b, qc = c // 4, c % 4
        xb = x[b].T.copy()  # [HID, T]
        xT_hi, xT_lo = _split8(xb, SX)
        qoff = qc * TQ
        in_maps.append(
            {
                "xT_hi": _pack_f8(xT_hi, 256).reshape(P, 8, 16, 256),
                "xT_lo": _pack_f8(xT_lo, 256).reshape(P, 8, 16, 256),
                "xq_hi": _pack_f8(xT_hi[:, qoff : qoff + TQ], TQ).reshape(P, 16, TQ),
                "xq_lo": _pack_f8(xT_lo[:, qoff : qoff + TQ], TQ).reshape(P, 16, TQ),
                "qa_hi": qa_hi,
                "qa_lo": qa_lo,
                "qb_hi": qb_hi,
                "qb_lo": qb_lo,
                "kva_hi": kva_hi,
                "kva_lo": kva_lo,
                "kvb_hi": kvb_hi,
                "kvb_lo": kvb_lo,
                "ow_hi": ow_hi,
                "ow_lo": ow_lo,
                "cosq": np.ascontiguousarray(cosq_full[:, qoff : qoff + TQ]),
                "sinq": np.ascontiguousarray(sinq_full[:, qoff : qoff + TQ]),
                "cosk": cosk,
                "sink": sink,
                "ones_in": ones,
                "ones32_in": ones32,
                "consts": consts,
            }
        )
    return in_maps


def _pack_f8(a, c):
    """pack for fp8 arrays without fp32 roundtrip (dtype-preserving)."""
    return _pack(a.view(np.uint8), c).view(F8NP)


def get_nc():
    if "nc" not in _CACHE:
        _CACHE["nc"] = _build_nc()
    return _CACHE["nc"]


def kernel(**inputs) -> np.ndarray:
    from concourse.bass_utils import run_bass_kernel_spmd

    nc = get_nc()
    in_maps = _host_prep(inputs)
    res = run_bass_kernel_spmd(nc, in_maps, core_ids=list(range(NCORES)))
    _CACHE["last_result"] = res
    outs = [res.results[c]["out"] for c in range(NCORES)]
    full = np.stack(
        [np.concatenate([outs[b * 4 + qc] for qc in range(4)], axis=0) for b in range(B)]
    )
    return full.astype(np.float32)
